# revision 1
# baseline (speedup 1.0000x reference)
"""Trainium2 Bass kernel for DecoupledAttentionAggregation GNN message passing.

Strategy (per sharding hint): destination nodes are dealt round-robin (after a
global degree-profile sort) across 8 cores; each core owns all edges into its
nodes, does local segment softmax / top-k / 3-group aggregation, and writes its
own output rows. The host shards/permutes/packs inputs; the device does all the
math (matmuls, softmax, top-k, weighted aggregation).

Device layout: each core's nodes are arranged into 128-row blocks. A node-row
r lives on SBUF partition r%128; its (per-group padded) edge slots occupy
consecutive f-columns of its block; blocks in a chunk share one width W, so
per-destination softmax/top-k are free-dim windowed ops with a single batched
access pattern. The 3 label-group aggregations are PSUM-accumulated identity
matmuls over each group's (uniform per block) column range.

Per-edge compute: one f-column x 128 partitions = a tile of 128 edge slots.
The host pre-stacks the per-slot operands feature-major so each tile needs 3
matmuls with data-stationary lhsT:
  A = [h_hi(64) | ea_hi(32) | ea_lo(32)]  (x2 weight sets)
  B = [h_lo(64) | ones(1)]                (x1 weight set + bias row)
giving fp32-accurate h@whW + ea@weW + b and the per-edge attention scores
(split-bf16 products; the dropped lo*lo terms are ~2^-18 relative).
"""

import sys

sys.path.insert(0, "/opt/trn_rl_repo")

import numpy as np
import ml_dtypes

import concourse.bacc as bacc
import concourse.bass as bass
import concourse.mybir as mybir
import concourse.tile as tile
from concourse import bass_utils

BF16 = mybir.dt.bfloat16
F32 = mybir.dt.float32

NCORES = 8
TOPK = 10
NEG = -1.0e30
H = 64
ED = 32
NH = 4
CHUNK_COLS = 96  # max f-columns per chunk (x128 slots)
PRECISION = "bf16"  # "bf16" (fast) or "f32" (fp32 messages + aggregation)
GPSIMD_WMSG_FRAC = 0.4  # fraction of the alpha*msg multiply offloaded to GPSIMD


def _bf16_split(x):
    hi = x.astype(ml_dtypes.bfloat16)
    lo = (x.astype(np.float32) - hi.astype(np.float32)).astype(ml_dtypes.bfloat16)
    return hi, lo


def _plan_and_pack(h, edge_index, edge_attr, node_labels, attn_w, whW, whb, weW, web):
    """Host-side sharding/packing. Returns (plan, in_maps, assemble_info)."""
    N = h.shape[0]
    row = np.asarray(edge_index[0], dtype=np.int64)
    col = np.asarray(edge_index[1], dtype=np.int64)
    labels = np.asarray(node_labels)

    # edge groups: 0=same, 1=diff, 2=unlabeled
    lr, lc = labels[row], labels[col]
    g = np.where(
        (lr == lc) & (lr != -1),
        0,
        np.where((lr != lc) & (lr != -1) & (lc != -1), 1, 2),
    ).astype(np.int64)

    deg_g = np.zeros((N, 3), np.int64)
    np.add.at(deg_g, (col, g), 1)

    # Global sort nodes by per-group degree profile, deal round-robin to cores.
    perm_global = np.lexsort((-deg_g[:, 2], -deg_g[:, 1], -deg_g[:, 0]))
    D = (N + NCORES - 1) // NCORES
    NB = (D + 127) // 128
    R = NB * 128

    node_of_row = np.full((NCORES, R), -1, np.int64)
    for c in range(NCORES):
        nodes_c = perm_global[c::NCORES]
        node_of_row[c, : len(nodes_c)] = nodes_c

    # canonical per-block per-group widths (max over cores, rounded to even)
    dg_rows = np.zeros((NCORES, R, 3), np.int64)
    for c in range(NCORES):
        valid = node_of_row[c] >= 0
        dg_rows[c, valid] = deg_g[node_of_row[c, valid]]
    Wg = dg_rows.reshape(NCORES, NB, 128, 3).max(axis=(0, 2))  # [NB,3]
    Wtot = Wg.sum(1)

    # Reorder blocks by Wtot desc so chunks have uniform width.
    border = np.argsort(-Wtot, kind="stable")
    Wg = Wg[border]
    Wtot = Wtot[border]
    rowperm = (border[:, None] * 128 + np.arange(128)[None, :]).reshape(-1)
    node_of_row = node_of_row[:, rowperm]

    # chunks: greedy fill; every block padded (in group 2) to the chunk width
    chunks = []
    b0 = 0
    while b0 < NB:
        Wc = int(Wtot[b0])
        if Wc == 0:
            break
        nmax = max(1, CHUNK_COLS // max(Wc, 1))
        b1 = min(b0 + nmax, NB)
        while b1 > b0 + 1 and Wtot[b1 - 1] == 0:
            b1 -= 1
        chunks.append((b0, b1, Wc))
        b0 = b1
    Wg = Wg.copy()
    for (b0, b1, Wc) in chunks:
        Wg[b0:b1, 2] += Wc - Wtot[b0:b1]
    Wtot = Wg.sum(1)
    Fb_off = np.concatenate([[0], np.cumsum(Wtot)])
    F = int(Fb_off[-1])

    in_maps = [dict() for _ in range(NCORES)]

    core_of_node = np.empty(N, np.int64)
    row_of_node = np.empty(N, np.int64)
    for c in range(NCORES):
        valid = node_of_row[c] >= 0
        core_of_node[node_of_row[c, valid]] = c
        row_of_node[node_of_row[c, valid]] = np.nonzero(valid)[0]

    e_core = core_of_node[col]
    e_row = row_of_node[col]

    # weights
    aw = np.asarray(attn_w, np.float32) * 0.25  # fold mean over heads
    a_r, a_c, a_e = aw[:H], aw[H : 2 * H], aw[2 * H :]
    whW = np.asarray(whW, np.float32)
    weW = np.asarray(weW, np.float32)
    bias_comb = np.asarray(whb, np.float32) + np.asarray(web, np.float32)

    W_hi, W_lo = _bf16_split(whW)
    ar_hi, ar_lo = _bf16_split(a_r)
    we_hi, we_lo = _bf16_split(weW)
    ae_hi, ae_lo = _bf16_split(a_e)
    ac_hi, ac_lo = _bf16_split(a_c)

    bf = ml_dtypes.bfloat16
    zED = np.zeros((ED, H + NH), np.float32)
    # A = [h_hi | ea_hi | ea_lo]; B = [h_lo | ones]
    WA1 = np.concatenate(
        [
            np.concatenate([W_hi, ar_hi], 1),
            np.concatenate([we_hi, ae_hi], 1),
            np.concatenate([we_hi, ae_hi], 1),
        ],
        0,
    ).astype(bf)  # [128, 68]
    WA2 = np.concatenate(
        [
            np.concatenate([W_lo, ar_lo], 1),
            np.concatenate([we_lo, ae_lo], 1),
            zED,
        ],
        0,
    ).astype(bf)
    WB = np.concatenate(
        [
            np.concatenate([W_hi, ar_hi], 1),
            np.concatenate([bias_comb[None, :], np.zeros((1, NH), np.float32)], 1),
        ],
        0,
    ).astype(bf)  # [65, 68]
    C1 = np.concatenate([ac_hi, ac_hi], 0).astype(bf)  # [128, 4]
    C2 = np.concatenate([ac_lo, ac_lo], 0).astype(bf)
    ident = np.eye(128, dtype=bf)

    h32 = np.asarray(h, np.float32)
    h_hi, h_lo = _bf16_split(h32)
    ea32 = np.asarray(edge_attr, np.float32)
    ea_hi, ea_lo = _bf16_split(ea32)

    goff = np.zeros((NB, 4), np.int64)
    goff[:, 1] = Wg[:, 0]
    goff[:, 2] = Wg[:, 0] + Wg[:, 1]
    goff[:, 3] = Wtot

    e_p = e_row & 127

    # order edges by (core, row, group); position within run -> slot column
    es = np.lexsort((g, e_row, e_core))
    key = (e_core[es] * R + e_row[es]) * 4 + g[es]
    runs_start = np.r_[True, key[1:] != key[:-1]]
    run_id = np.cumsum(runs_start) - 1
    first_of = np.full(run_id[-1] + 1, len(es), np.int64)
    np.minimum.at(first_of, run_id, np.arange(len(es)))
    pos = np.arange(len(es)) - first_of[run_id]
    e_block = e_row >> 7
    fcol = Fb_off[e_block[es]] + goff[e_block[es], g[es]] + pos
    assert (pos < Wg[e_block[es], g[es]]).all()

    chunk_meta = []
    for (b0, b1, Wc) in chunks:
        chunk_meta.append(
            dict(
                b0=b0,
                b1=b1,
                cols=int(Fb_off[b1] - Fb_off[b0]),
                col_off=int(Fb_off[b0]),
                nrows=(b1 - b0) * 128,
                row_off=b0 * 128,
                W=int(Wc),
            )
        )

    # pre-transposed source tables with a trailing zero row/col for pads
    hT_hi = np.ascontiguousarray(np.concatenate([h_hi, np.zeros((1, H), bf)]).T.view(np.uint16))
    hT_lo = np.ascontiguousarray(np.concatenate([h_lo, np.zeros((1, H), bf)]).T.view(np.uint16))
    eaT_hi = np.ascontiguousarray(np.concatenate([ea_hi, np.zeros((1, ED), bf)]).T.view(np.uint16))
    eaT_lo = np.ascontiguousarray(np.concatenate([ea_lo, np.zeros((1, ED), bf)]).T.view(np.uint16))
    E = len(row)

    eid_grids = []
    for c in range(NCORES):
        mask = e_core[es] == c
        ef = es[mask]
        fc = fcol[mask]
        pp = e_p[ef]
        eid_grid = np.full((F, 128), -1, np.int64)
        eid_grid[fc, pp] = ef
        eid_grids.append(eid_grid)
        real = eid_grid >= 0
        flat_eid = eid_grid.reshape(-1)
        flat_real = real.reshape(-1)
        idxr = np.where(flat_real, flat_eid, E)  # E -> zero row
        rsrc = np.where(flat_real, row[np.maximum(flat_eid, 0)], N)  # N -> zero row

        A = np.empty((128, F * 128), bf)
        A16 = A.view(np.uint16)
        A16[:H, :] = hT_hi[:, rsrc]
        A16[H : H + ED, :] = eaT_hi[:, idxr]
        A16[H + ED :, :] = eaT_lo[:, idxr]
        B = np.empty((H + 1, F * 128), bf)
        B16 = B.view(np.uint16)
        B16[:H, :] = hT_lo[:, rsrc]
        B[H, :] = bf(1.0)
        sbias = np.where(real, 0.0, NEG).astype(np.float32).T.copy()  # [128,F]

        dn = np.where(node_of_row[c] >= 0, node_of_row[c], 0)
        hdT = np.concatenate([h_hi[dn], h_lo[dn]], 1).T.copy()  # [128, R] bf16

        m = in_maps[c]
        m["A"] = A
        m["B"] = B
        m["sbias"] = sbias
        m["hdT"] = np.ascontiguousarray(hdT)
        m["WA1"], m["WA2"], m["WB"] = WA1, WA2, WB
        m["C1"], m["C2"] = C1, C2
        m["ident"] = ident

    plan = dict(N=N, D=D, NB=NB, R=R, F=F, Wg=Wg, Wtot=Wtot, Fb_off=Fb_off,
                goff=goff, chunks=chunk_meta,
                has_bias=bool(np.any(bias_comb != 0)))
    assemble = dict(node_of_row=node_of_row, R=R, eid_grids=eid_grids)
    return plan, in_maps, assemble


def _build_program(plan, precision="bf16", debug=False):
    fp32 = precision == "f32"
    NB, F, R = plan["NB"], plan["F"], plan["R"]
    Fb_off = plan["Fb_off"]
    chunks = plan["chunks"]

    nc = bacc.Bacc(
        "TRN2",
        target_bir_lowering=False,
        debug=False,
        enable_asserts=False,
        num_devices=NCORES,
    )

    A_d = nc.dram_tensor("A", [128, F * 128], BF16, kind="ExternalInput")
    B_d = nc.dram_tensor("B", [H + 1, F * 128], BF16, kind="ExternalInput")
    sbias_d = nc.dram_tensor("sbias", [128, F], F32, kind="ExternalInput")
    hdT_d = nc.dram_tensor("hdT", [128, R], BF16, kind="ExternalInput")
    WA1_d = nc.dram_tensor("WA1", [128, H + NH], BF16, kind="ExternalInput")
    WA2_d = nc.dram_tensor("WA2", [128, H + NH], BF16, kind="ExternalInput")
    WB_d = nc.dram_tensor("WB", [H + 1, H + NH], BF16, kind="ExternalInput")
    C1_d = nc.dram_tensor("C1", [128, NH], BF16, kind="ExternalInput")
    C2_d = nc.dram_tensor("C2", [128, NH], BF16, kind="ExternalInput")
    id_d = nc.dram_tensor("ident", [128, 128], BF16, kind="ExternalInput")
    out_d = nc.dram_tensor("out", [R, 3 * H], F32, kind="ExternalOutput")
    if debug:
        dbg_s = nc.dram_tensor("dbg_s", [128, F], F32, kind="ExternalOutput")
        dbg_al = nc.dram_tensor("dbg_al", [128, F], F32, kind="ExternalOutput")
        dbg_msg = nc.dram_tensor("dbg_msg", [128, F * H], F32, kind="ExternalOutput")

    msg_dt = F32 if fp32 else BF16
    wm_dt = F32 if fp32 else BF16

    with tile.TileContext(nc) as tc:
        with (
            tc.tile_pool(name="const", bufs=1) as cpool,
            tc.tile_pool(name="dma", bufs=2) as dpool,
            tc.tile_pool(name="work", bufs=2) as wpool,
            tc.tile_pool(name="psum_m", bufs=3, space="PSUM") as pmpool,
            tc.tile_pool(name="psum_o", bufs=2, space="PSUM") as popool,
            tc.tile_pool(name="psum_h", bufs=2, space="PSUM") as phpool,
        ):
            WA1_s = cpool.tile([128, H + NH], BF16, tag="wa1")
            WA2_s = cpool.tile([128, H + NH], BF16, tag="wa2")
            WB_s = cpool.tile([H + 1, H + NH], BF16, tag="wb")
            C1_s = cpool.tile([128, NH], BF16, tag="c1")
            C2_s = cpool.tile([128, NH], BF16, tag="c2")
            id_s = cpool.tile([128, 128], BF16, tag="ident")
            for s, d in [(WA1_s, WA1_d), (WA2_s, WA2_d), (WB_s, WB_d),
                         (C1_s, C1_d), (C2_s, C2_d), (id_s, id_d)]:
                nc.sync.dma_start(out=s[:], in_=d.ap())

            for mi, cm in enumerate(chunks):
                cols = cm["cols"]
                nrows = cm["nrows"]
                nblk = nrows // 128
                Wc = cm["W"]
                c0 = cm["col_off"]
                nslots = cols * 128

                A_sb = dpool.tile([128, nslots], BF16, tag="A")
                nc.sync.dma_start(
                    out=A_sb[:], in_=A_d.ap()[:, c0 * 128 : c0 * 128 + nslots]
                )
                B_sb = dpool.tile([H + 1, nslots], BF16, tag="B")
                nc.sync.dma_start(
                    out=B_sb[:], in_=B_d.ap()[:, c0 * 128 : c0 * 128 + nslots]
                )
                bias_sb = dpool.tile([128, cols], F32, tag="bias")
                nc.sync.dma_start(out=bias_sb[:], in_=sbias_d.ap()[:, c0 : c0 + cols])
                hdT_sb = dpool.tile([128, nrows], BF16, tag="hdT")
                nc.sync.dma_start(
                    out=hdT_sb[:],
                    in_=hdT_d.ap()[:, cm["row_off"] : cm["row_off"] + nrows],
                )

                # destination-node score component: [128 rows, 4] per block
                hcs_sb = wpool.tile([128, nblk, NH], F32, tag="hcs")
                ph = phpool.tile([128, nblk * NH], F32, tag="psum_hc")
                for b in range(nblk):
                    sl = slice(b * 128, (b + 1) * 128)
                    po = ph[:, b * NH : (b + 1) * NH]
                    nc.tensor.matmul(out=po, lhsT=hdT_sb[:, sl], rhs=C1_s[:],
                                     start=True, stop=False)
                    nc.tensor.matmul(out=po, lhsT=hdT_sb[:, sl], rhs=C2_s[:],
                                     start=False, stop=True)
                nc.vector.tensor_copy(
                    out=hcs_sb[:], in_=ph[:].rearrange("p (b f) -> p b f", f=NH)
                )

                # per-edge message + score matmuls
                msg_sb = wpool.tile([128, cols, H], msg_dt, tag="msg")
                sraw_sb = wpool.tile([128, cols, NH], F32, tag="sraw")
                PB = 7
                for t0 in range(0, cols, PB):
                    tb = min(PB, cols - t0)
                    pm = pmpool.tile([128, PB * (H + NH)], F32, tag="psum_msg")
                    for j in range(tb):
                        t = t0 + j
                        sl = slice(t * 128, (t + 1) * 128)
                        po = pm[:, j * (H + NH) : (j + 1) * (H + NH)]
                        nc.tensor.matmul(out=po, lhsT=A_sb[:, sl], rhs=WA1_s[:],
                                         start=True, stop=False)
                        nc.tensor.matmul(out=po[:, H:], lhsT=A_sb[:, sl],
                                         rhs=WA2_s[:, H:],
                                         start=False, stop=False)
                        if plan["has_bias"]:
                            nc.tensor.matmul(out=po, lhsT=B_sb[:, sl], rhs=WB_s[:],
                                             start=False, stop=True)
                        else:
                            nc.tensor.matmul(out=po[:, H:], lhsT=B_sb[:, sl],
                                             rhs=WB_s[:, H:],
                                             start=False, stop=True)
                    pmv = pm[:].rearrange("p (t f) -> p t f", f=H + NH)
                    nc.scalar.activation(
                        out=msg_sb[:, t0 : t0 + tb, :],
                        in_=pmv[:, :tb, :H],
                        func=mybir.ActivationFunctionType.Relu,
                    )
                    nc.vector.tensor_copy(
                        out=sraw_sb[:, t0 : t0 + tb, :], in_=pmv[:, :tb, H:]
                    )

                # scores: add dest component, leaky-relu, head-sum, pad bias
                srawW = sraw_sb[:].rearrange("p (b w) f -> p b w f", w=Wc)
                nc.vector.tensor_tensor(
                    out=srawW, in0=srawW,
                    in1=hcs_sb[:].unsqueeze(2).to_broadcast([128, nblk, Wc, NH]),
                    op=mybir.AluOpType.add,
                )
                lr_sb = wpool.tile([128, cols, NH], F32, tag="lrelu")
                nc.vector.tensor_scalar_mul(lr_sb[:], sraw_sb[:], 0.2)
                nc.vector.tensor_tensor(out=sraw_sb[:], in0=sraw_sb[:], in1=lr_sb[:],
                                        op=mybir.AluOpType.max)
                s_sb = wpool.tile([128, cols], F32, tag="scores")
                nc.vector.tensor_reduce(out=s_sb[:], in_=sraw_sb[:],
                                        axis=mybir.AxisListType.X,
                                        op=mybir.AluOpType.add)
                nc.vector.tensor_tensor(out=s_sb[:], in0=s_sb[:], in1=bias_sb[:],
                                        op=mybir.AluOpType.add)
                ex_sb = wpool.tile([128, cols], F32, tag="ex")
                nc.scalar.activation(out=ex_sb[:], in_=s_sb[:],
                                     func=mybir.ActivationFunctionType.Exp)
                sW = lambda ap: ap.rearrange("p (b w) -> p b w", w=Wc)
                den_sb = wpool.tile([128, nblk], F32, tag="den")
                nc.vector.tensor_reduce(out=den_sb[:], in_=sW(ex_sb[:]),
                                        axis=mybir.AxisListType.X,
                                        op=mybir.AluOpType.add)
                nc.vector.tensor_scalar_add(den_sb[:], den_sb[:], 1e-30)
                inv_sb = wpool.tile([128, nblk], F32, tag="invden")
                nc.vector.reciprocal(out=inv_sb[:], in_=den_sb[:])

                # top-k threshold: iterative max extraction on a copy of ex
                work_sb = wpool.tile([128, cols], F32, tag="work")
                nc.vector.tensor_copy(out=work_sb[:], in_=ex_sb[:])
                m_sb = wpool.tile([128, nblk], F32, tag="mx")
                tmp_sb = wpool.tile([128, cols], F32, tag="tmp")
                mbc = m_sb[:].unsqueeze(2).to_broadcast([128, nblk, Wc])
                for it in range(TOPK):
                    nc.vector.tensor_reduce(out=m_sb[:], in_=sW(work_sb[:]),
                                            axis=mybir.AxisListType.X,
                                            op=mybir.AluOpType.max)
                    if it < TOPK - 1:
                        nc.vector.tensor_tensor(out=sW(tmp_sb[:]), in0=sW(work_sb[:]),
                                                in1=mbc,
                                                op=mybir.AluOpType.not_equal)
                        nc.vector.tensor_tensor(out=work_sb[:], in0=work_sb[:],
                                                in1=tmp_sb[:],
                                                op=mybir.AluOpType.mult)

                # alpha = ex * (ex >= theta) * inv_den
                al_sb = wpool.tile([128, cols], F32, tag="alpha")
                nc.vector.tensor_tensor(out=sW(al_sb[:]), in0=sW(ex_sb[:]), in1=mbc,
                                        op=mybir.AluOpType.is_ge)
                nc.vector.tensor_tensor(out=al_sb[:], in0=al_sb[:], in1=ex_sb[:],
                                        op=mybir.AluOpType.mult)
                ibc = inv_sb[:].unsqueeze(2).to_broadcast([128, nblk, Wc])
                nc.vector.tensor_tensor(out=sW(al_sb[:]), in0=sW(al_sb[:]), in1=ibc,
                                        op=mybir.AluOpType.mult)

                # weighted messages (split DVE / GPSIMD)
                wmsg_sb = wpool.tile([128, cols, H], wm_dt, tag="wmsg")
                csplit = int(cols * (1.0 - GPSIMD_WMSG_FRAC)) & ~1
                abc = al_sb[:].unsqueeze(2).to_broadcast([128, cols, H])
                nc.vector.tensor_tensor(
                    out=wmsg_sb[:, :csplit, :], in0=msg_sb[:, :csplit, :],
                    in1=abc[:, :csplit, :], op=mybir.AluOpType.mult)
                if csplit < cols:
                    nc.gpsimd.tensor_tensor(
                        out=wmsg_sb[:, csplit:, :], in0=msg_sb[:, csplit:, :],
                        in1=abc[:, csplit:, :], op=mybir.AluOpType.mult)

                if debug:
                    nc.sync.dma_start(out=dbg_s.ap()[:, c0 : c0 + cols], in_=s_sb[:])
                    nc.sync.dma_start(out=dbg_al.ap()[:, c0 : c0 + cols], in_=al_sb[:])
                    dmsg = wpool.tile([128, cols, H], F32, tag="dbgmsg")
                    nc.vector.tensor_copy(out=dmsg[:], in_=msg_sb[:])
                    nc.sync.dma_start(
                        out=dbg_msg.ap()[:, c0 * H : (c0 + cols) * H], in_=dmsg[:]
                    )

                # aggregation per block/group (PSUM-accumulated identity matmuls)
                for b in range(nblk):
                    gb = plan["Wg"][cm["b0"] + b]
                    bc0 = int(Fb_off[cm["b0"] + b] - c0)
                    po = popool.tile([128, 3 * H], F32, tag="psum_out")
                    osb = wpool.tile([128, 3 * H], F32, tag="outsb")
                    off = 0
                    for gi in range(3):
                        wgi = int(gb[gi])
                        if wgi == 0:
                            off += wgi
                            continue
                        if fp32:
                            nc.vector.tensor_reduce(
                                out=osb[:, gi * H : (gi + 1) * H],
                                in_=wmsg_sb[:, bc0 + off : bc0 + off + wgi, :]
                                .rearrange("p w f -> p f w"),
                                axis=mybir.AxisListType.X,
                                op=mybir.AluOpType.add,
                            )
                        else:
                            for j in range(wgi):
                                nc.tensor.matmul(
                                    out=po[:, gi * H : (gi + 1) * H],
                                    lhsT=id_s[:],
                                    rhs=wmsg_sb[:, bc0 + off + j, :],
                                    start=(j == 0),
                                    stop=(j == wgi - 1),
                                )
                        off += wgi
                    if not fp32:
                        nc.vector.tensor_copy(out=osb[:], in_=po[:])
                    for gi in range(3):
                        if int(gb[gi]) == 0:
                            nc.vector.memset(osb[:, gi * H : (gi + 1) * H], 0.0)
                    nc.sync.dma_start(
                        out=out_d.ap()[
                            cm["row_off"] + b * 128 : cm["row_off"] + (b + 1) * 128, :
                        ],
                        in_=osb[:],
                    )

    nc.compile()
    return nc


_LAST = {}


def kernel(**inputs):
    import time

    t0 = time.time()
    h = np.asarray(inputs["h"])
    plan, in_maps, assemble = _plan_and_pack(
        h,
        np.asarray(inputs["edge_index"]),
        np.asarray(inputs["edge_attr"]),
        np.asarray(inputs["node_labels"]),
        np.asarray(inputs["attn_w"]),
        np.asarray(inputs["whW"]),
        np.asarray(inputs["whb"]),
        np.asarray(inputs["weW"]),
        np.asarray(inputs["web"]),
    )
    t1 = time.time()
    nc = _build_program(plan, precision=PRECISION)
    t2 = time.time()
    _LAST.update(nc=nc, in_maps=in_maps, plan=plan, assemble=assemble)
    res = bass_utils.run_bass_kernel_spmd(nc, in_maps, core_ids=list(range(NCORES)))
    t3 = time.time()
    print(f"kernel phases: pack {t1-t0:.1f}s build+compile {t2-t1:.1f}s run {t3-t2:.1f}s",
          flush=True)
    N = plan["N"]
    out = np.zeros((N, 3 * H), np.float32)
    nr = assemble["node_of_row"]
    for c in range(NCORES):
        o = res.results[c]["out"]
        valid = nr[c] >= 0
        out[nr[c, valid]] = o[valid]
    return out

